# revision 2
# baseline (speedup 1.0000x reference)
"""Trainium2 Bass kernel for nn_AdditiveAttention (B=8, C=128, H=W=64).

Reference computation (per batch b):
    xf = x.reshape(C, N)                      # N = H*W = 4096
    Q  = Wq @ xf + bq                         # [D, N], D=16
    K  = Wk @ xf + bk                         # [D, N]
    V  = Wv @ xf + bv                         # [C, N]
    E  = tanh(Wm^T @ Q + b)                   # [D, N]  (energy pre-softmax, row n: E^T[n, :])
    energy[n, m] = sum_e E[e, n] * K[e, m]    # [N, N]
    att = softmax(energy, axis=-1)            # softmax over m
    out[c, n] = sum_m V[c, m] * att[n, m]     # = V @ att^T
    y = gamma * out + x

Math notes used by the kernel:
  - bk adds a per-e constant to K; its contribution to energy is constant along
    the softmax axis and cancels exactly in softmax -> ignored.
  - bq and b fold into a per-e bias inside tanh: ce = Wm^T @ bq + b.
  - Wq then folds with Wm: E = tanh((Wm^T Wq) @ xf + ce).
  - bv folds out of the attention matmul: out = out_raw / S + bv, so the
    kernel computes with raw V and adds gamma*bv at the end.
  - exp is computed without max subtraction: |energy| <= D * max|K| ~ 21 for
    these magnitudes, so exp stays well inside fp32/bf16 range.

Device layout (one batch per NeuronCore, 8 cores, no collectives):
  attP[j, i] = exp(energy[i, j]) is produced in column layout (partition = j =
  softmax axis) so it can feed the output matmul directly as the moving
  operand; the softmax denominators S[i] (partition-axis sums) are built by
  accumulating the exp tiles on the vector engine (bf16) and reducing with a
  ones-vector matmul on the tensor engine; normalization happens after the
  output matmul on the 128x512 result instead of on the 4096x4096 attention.
"""

import numpy as np

import concourse.bacc as bacc
import concourse.bass as bass
import concourse.mybir as mybir
import concourse.tile as tile
from concourse import bass_utils

F32 = mybir.dt.float32
BF16 = mybir.dt.bfloat16

B, C, HH, WW = 8, 128, 64, 64
N = HH * WW  # 4096
D = 16
N_CORES = 8

IC = 512          # columns of the output / softmax-row chunk
GJT = 2           # j-tiles (of 128) per exp group; group free dim = GJT*512


def build_nc(n_tok: int = N, debug: bool = False):
    """Build the per-core Bass program. n_tok is the token count (4096 for the
    real problem; smaller for simulator checks). Must be a multiple of 1024."""
    assert n_tok % 1024 == 0
    n_ic = n_tok // IC            # output column chunks
    n_jt = n_tok // 128           # 128-row j tiles
    n_g = n_jt // GJT             # exp groups per ic

    nc = bacc.Bacc("TRN2", target_bir_lowering=False, debug=debug)

    x_d = nc.dram_tensor("x", [C, n_tok], F32, kind="ExternalInput")
    wkT_d = nc.dram_tensor("wkT", [C, D], F32, kind="ExternalInput")
    wqm_d = nc.dram_tensor("wqm", [C, D], F32, kind="ExternalInput")
    wvT_d = nc.dram_tensor("wvT", [C, C], F32, kind="ExternalInput")
    ce_d = nc.dram_tensor("ce", [D, 1], F32, kind="ExternalInput")
    gamma_d = nc.dram_tensor("gamma", [1, 1], F32, kind="ExternalInput")
    gbv_d = nc.dram_tensor("gbv", [C, 1], F32, kind="ExternalInput")
    out_d = nc.dram_tensor("out", [C, n_tok], F32, kind="ExternalOutput")

    with tile.TileContext(nc) as tc:
        with (
            tc.tile_pool(name="const", bufs=1) as const,
            tc.tile_pool(name="big", bufs=1) as big,
            tc.tile_pool(name="work", bufs=3) as work,
            tc.tile_pool(name="acc", bufs=2) as acc,
            tc.tile_pool(name="small", bufs=2) as small,
            tc.tile_pool(name="psum_e", bufs=2, space=bass.MemorySpace.PSUM) as psum_e,
            tc.tile_pool(name="psum_o", bufs=2, space=bass.MemorySpace.PSUM) as psum_o,
            tc.tile_pool(name="psum_s", bufs=2, space=bass.MemorySpace.PSUM) as psum_s,
        ):
            # ---- constants ----
            wkT = const.tile([C, D], F32)
            wqm = const.tile([C, D], F32)
            wvT = const.tile([C, C], F32)
            ce = const.tile([D, 1], F32)
            gamma = const.tile([1, 1], F32)
            gbv = const.tile([C, 1], F32)
            ones_bf = const.tile([C, 1], BF16)
            nc.sync.dma_start(wkT[:], wkT_d[:])
            nc.sync.dma_start(wqm[:], wqm_d[:])
            nc.sync.dma_start(wvT[:], wvT_d[:])
            nc.sync.dma_start(ce[:], ce_d[:])
            nc.sync.dma_start(gamma[:], gamma_d[:])
            nc.sync.dma_start(gbv[:], gbv_d[:])
            nc.vector.memset(ones_bf[:], 1.0)

            # ---- x ----
            xt = big.tile([C, n_tok], F32, tag="x")
            nc.sync.dma_start(xt[:], x_d[:])

            # ---- projections ----
            K_sb = big.tile([D, n_tok], F32, tag="K")
            E_sb = big.tile([D, n_tok], F32, tag="E")
            Vt_sb = big.tile([C, n_tok], BF16, tag="Vt")

            for ch in range(n_tok // 512):
                eps = psum_e.tile([128, GJT * 512], F32, tag="eps")
                xc = xt[:, ch * 512:(ch + 1) * 512]
                # K = Wk @ x  (raw, bias dropped: cancels in softmax)
                nc.tensor.matmul(eps[0:D, 0:512], wkT[:], xc, start=True, stop=True)
                nc.vector.tensor_copy(K_sb[:, ch * 512:(ch + 1) * 512], eps[0:D, 0:512])
                # E = tanh((Wm^T Wq) @ x + ce)
                nc.tensor.matmul(eps[0:D, 512:1024], wqm[:], xc, start=True, stop=True)
                nc.scalar.activation(
                    E_sb[:, ch * 512:(ch + 1) * 512],
                    eps[0:D, 512:1024],
                    mybir.ActivationFunctionType.Tanh,
                    bias=ce[:, 0:1],
                    scale=1.0,
                )

            # Vt[n, c] = sum_ch x[ch, n] * Wv[c, ch]  (V transposed, raw)
            for ch in range(n_tok // 512):
                vps = psum_e.tile([128, GJT * 512], F32, tag="eps")
                for k in range(4):
                    jt = ch * 4 + k
                    nc.tensor.matmul(
                        vps[:, k * 128:(k + 1) * 128],
                        xt[:, jt * 128:(jt + 1) * 128],
                        wvT[:],
                        start=True,
                        stop=True,
                    )
                nc.vector.tensor_copy(Vt_sb[:, ch * 512:(ch + 1) * 512], vps[:, 0:512])

            # ---- main attention loop ----
            for ic in range(n_ic):
                i0 = ic * IC
                out_ps = psum_o.tile([128, IC], F32, tag="o")
                s_ps = psum_s.tile([1, IC], F32, tag="s")
                T0 = acc.tile([128, IC], BF16, tag="T0")
                T1 = acc.tile([128, IC], BF16, tag="T1")
                nc.vector.memset(T0[:], 0.0)
                nc.vector.memset(T1[:], 0.0)

                for g in range(n_g):
                    eps = psum_e.tile([128, GJT * 512], F32, tag="eps")
                    xp = work.tile([128, GJT * 512], BF16, tag="xp")
                    for r in range(GJT):
                        jt = g * GJT + r
                        # energyP[j, i] = sum_e K[e, j] * E[e, i]
                        nc.tensor.matmul(
                            eps[:, r * 512:(r + 1) * 512],
                            K_sb[:, jt * 128:(jt + 1) * 128],
                            E_sb[:, i0:i0 + IC],
                            start=True,
                            stop=True,
                        )
                    nc.scalar.activation(
                        xp[:], eps[:], mybir.ActivationFunctionType.Exp
                    )
                    for r in range(GJT):
                        jt = g * GJT + r
                        # out_raw[c, i] += sum_j Vt[j, c] * expP[j, i]
                        nc.tensor.matmul(
                            out_ps[:],
                            Vt_sb[:, jt * 128:(jt + 1) * 128],
                            xp[:, r * 512:(r + 1) * 512],
                            start=(g == 0 and r == 0),
                            stop=(g == n_g - 1 and r == GJT - 1),
                        )
                        # denominator partial sums (bf16)
                        Tg = T0 if r == 0 else T1
                        nc.vector.tensor_add(
                            Tg[:], Tg[:], xp[:, r * 512:(r + 1) * 512]
                        )

                # S[i] = sum_j T0[j, i] + T1[j, i] via ones matmul
                nc.tensor.matmul(s_ps[:], ones_bf[:], T0[:], start=True, stop=False)
                nc.tensor.matmul(s_ps[:], ones_bf[:], T1[:], start=False, stop=True)

                sr = small.tile([1, IC], F32, tag="sr")
                nc.vector.reciprocal(sr[:], s_ps[:])
                nc.vector.tensor_scalar_mul(sr[:], sr[:], gamma[:, 0:1])
                bc = small.tile([128, IC], F32, tag="bc")
                sr_ap = sr[:]
                sr_bcast = bass.AP(
                    tensor=sr_ap.tensor,
                    offset=sr_ap.offset,
                    ap=[[1, 1], [0, 128], [1, IC]],
                )
                nc.gpsimd.dma_start(bc[:], sr_bcast)
                # z = out_raw * (gamma / S)
                z = small.tile([128, IC], F32, tag="z")
                nc.vector.scalar_tensor_tensor(
                    z[:], out_ps[:], 0.0, bc[:],
                    op0=mybir.AluOpType.add, op1=mybir.AluOpType.mult,
                )
                # y = z + gamma*bv + x
                zf = small.tile([128, IC], F32, tag="zf")
                nc.vector.scalar_tensor_tensor(
                    zf[:], z[:], gbv[:, 0:1], xt[:, i0:i0 + IC],
                    op0=mybir.AluOpType.add, op1=mybir.AluOpType.add,
                )
                nc.sync.dma_start(out_d[:, i0:i0 + IC], zf[:])

    nc.compile()
    return nc


_NC_CACHE: dict = {}


def _get_nc(n_tok: int = N):
    if n_tok not in _NC_CACHE:
        _NC_CACHE[n_tok] = build_nc(n_tok)
    return _NC_CACHE[n_tok]


def make_in_maps(x, Wq, bq, Wk, bk, Wv, bv, Wm, b, gamma, n_tok: int = N):
    """Host-side prep: tiny weight transforms + per-core sharding (batch b -> core b)."""
    x = np.ascontiguousarray(np.asarray(x, np.float32)).reshape(B, C, n_tok)
    Wq = np.asarray(Wq, np.float32)
    Wk = np.asarray(Wk, np.float32)
    Wv = np.asarray(Wv, np.float32)
    Wm = np.asarray(Wm, np.float32)
    bqv = np.asarray(bq, np.float32)
    bv_ = np.asarray(bv, np.float32)
    bs = np.asarray(b, np.float32)
    gm = np.asarray(gamma, np.float32)

    wkT = np.ascontiguousarray(Wk.T)                    # [C, D]
    wqm = np.ascontiguousarray(Wq.T @ Wm)               # [C, D]
    wvT = np.ascontiguousarray(Wv.T)                    # [C, C]
    ce = (Wm.T @ bqv + bs[0]).reshape(D, 1)             # [D, 1]
    gmat = gm.reshape(1, 1)
    gbv = (gm[0] * bv_).reshape(C, 1)

    common = {
        "wkT": wkT, "wqm": wqm, "wvT": wvT,
        "ce": np.ascontiguousarray(ce, np.float32),
        "gamma": np.ascontiguousarray(gmat, np.float32),
        "gbv": np.ascontiguousarray(gbv, np.float32),
    }
    return [{"x": np.ascontiguousarray(x[core]), **common} for core in range(B)]


def kernel(**inputs) -> np.ndarray:
    nc = _get_nc(N)
    in_maps = make_in_maps(**inputs)
    res = bass_utils.run_bass_kernel_spmd(
        nc, in_maps, core_ids=list(range(N_CORES))
    )
    out = np.stack([res.results[core]["out"] for core in range(B)])
    return out.reshape(B, C, HH, WW).astype(np.float32)


# revision 3
# speedup vs baseline: 1.9076x; 1.9076x over previous
"""Trainium2 Bass kernel for nn_AdditiveAttention (B=8, C=128, H=W=64).

Reference computation (per batch b):
    xf = x.reshape(C, N)                      # N = H*W = 4096
    Q  = Wq @ xf + bq                         # [D, N], D=16
    K  = Wk @ xf + bk                         # [D, N]
    V  = Wv @ xf + bv                         # [C, N]
    E  = tanh(Wm^T @ Q + b)                   # [D, N]  (energy pre-softmax, row n: E^T[n, :])
    energy[n, m] = sum_e E[e, n] * K[e, m]    # [N, N]
    att = softmax(energy, axis=-1)            # softmax over m
    out[c, n] = sum_m V[c, m] * att[n, m]     # = V @ att^T
    y = gamma * out + x

Math notes used by the kernel:
  - bk adds a per-e constant to K; its contribution to energy is constant along
    the softmax axis and cancels exactly in softmax -> ignored.
  - bq and b fold into a per-e bias inside tanh: ce = Wm^T @ bq + b.
  - Wq then folds with Wm: E = tanh((Wm^T Wq) @ xf + ce).
  - bv folds out of the attention matmul: out = out_raw / S + bv, so the
    kernel computes with raw V and adds gamma*bv at the end.
  - exp is computed without max subtraction: |energy| <= D * max|K| ~ 21 for
    these magnitudes, so exp stays well inside fp32/bf16 range.

Device layout (one batch per NeuronCore, 8 cores, no collectives):
  attP[j, i] = exp(energy[i, j]) is produced in column layout (partition = j =
  softmax axis) so it can feed the output matmul directly as the moving
  operand; the softmax denominators S[i] (partition-axis sums) are built by
  accumulating the exp tiles on the vector engine (bf16) and reducing with a
  ones-vector matmul on the tensor engine; normalization happens after the
  output matmul on the 128x512 result instead of on the 4096x4096 attention.
"""

import numpy as np

import concourse.bacc as bacc
import concourse.bass as bass
import concourse.mybir as mybir
import concourse.tile as tile
from concourse import bass_utils

F32 = mybir.dt.float32
BF16 = mybir.dt.bfloat16

B, C, HH, WW = 8, 128, 64, 64
N = HH * WW  # 4096
D = 16
N_CORES = 8

IC = 512          # columns of the output / softmax-row chunk
GJT = 2           # j-tiles (of 128) per exp group; group free dim = GJT*512


def build_nc(n_tok: int = N, debug: bool = False):
    """Build the per-core Bass program. n_tok is the token count (4096 for the
    real problem; smaller for simulator checks). Must be a multiple of 1024."""
    assert n_tok % 1024 == 0
    n_ic = n_tok // IC            # output column chunks
    n_jt = n_tok // 128           # 128-row j tiles
    n_g = n_jt // GJT             # exp groups per ic

    nc = bacc.Bacc("TRN2", target_bir_lowering=False, debug=debug)

    x_d = nc.dram_tensor("x", [C, n_tok], F32, kind="ExternalInput")
    wkT_d = nc.dram_tensor("wkT", [C, D], BF16, kind="ExternalInput")
    wqm_d = nc.dram_tensor("wqm", [C, D], BF16, kind="ExternalInput")
    wvT_d = nc.dram_tensor("wvT", [C, C], BF16, kind="ExternalInput")
    ce_d = nc.dram_tensor("ce", [D, 1], F32, kind="ExternalInput")
    gamma_d = nc.dram_tensor("gamma", [1, 1], F32, kind="ExternalInput")
    gbv_d = nc.dram_tensor("gbv", [C, 1], F32, kind="ExternalInput")
    out_d = nc.dram_tensor("out", [C, n_tok], F32, kind="ExternalOutput")

    with tile.TileContext(nc) as tc:
        with (
            tc.tile_pool(name="const", bufs=1) as const,
            tc.tile_pool(name="big", bufs=1) as big,
            tc.tile_pool(name="work", bufs=3) as work,
            tc.tile_pool(name="acc", bufs=2) as acc,
            tc.tile_pool(name="small", bufs=2) as small,
            tc.tile_pool(name="psum_e", bufs=2, space=bass.MemorySpace.PSUM) as psum_e,
            tc.tile_pool(name="psum_o", bufs=2, space=bass.MemorySpace.PSUM) as psum_o,
            tc.tile_pool(name="psum_s", bufs=2, space=bass.MemorySpace.PSUM) as psum_s,
        ):
            # ---- constants ----
            wkT = const.tile([C, D], BF16)
            wqm = const.tile([C, D], BF16)
            wvT = const.tile([C, C], BF16)
            ce = const.tile([D, 1], F32)
            gamma = const.tile([1, 1], F32)
            gbv = const.tile([C, 1], F32)
            ones_bf = const.tile([C, 1], BF16)
            nc.sync.dma_start(wkT[:], wkT_d[:])
            nc.sync.dma_start(wqm[:], wqm_d[:])
            nc.sync.dma_start(wvT[:], wvT_d[:])
            nc.sync.dma_start(ce[:], ce_d[:])
            nc.sync.dma_start(gamma[:], gamma_d[:])
            nc.sync.dma_start(gbv[:], gbv_d[:])
            nc.vector.memset(ones_bf[:], 1.0)

            # ---- x ----
            xt = big.tile([C, n_tok], F32, tag="x")
            nc.sync.dma_start(xt[:], x_d[:])
            x_bf = big.tile([C, n_tok], BF16, tag="xbf")
            nc.vector.tensor_copy(x_bf[:], xt[:])

            # ---- projections ----
            K_sb = big.tile([D, n_tok], BF16, tag="K")
            E_sb = big.tile([D, n_tok], BF16, tag="E")
            Vt_sb = big.tile([C, n_tok], BF16, tag="Vt")

            for ch in range(n_tok // 512):
                eps = psum_e.tile([128, GJT * 512], F32, tag="eps")
                xc = x_bf[:, ch * 512:(ch + 1) * 512]
                # K = Wk @ x  (raw, bias dropped: cancels in softmax)
                nc.tensor.matmul(eps[0:D, 0:512], wkT[:], xc, start=True, stop=True)
                nc.vector.tensor_copy(K_sb[:, ch * 512:(ch + 1) * 512], eps[0:D, 0:512])
                # E = tanh((Wm^T Wq) @ x + ce)
                nc.tensor.matmul(eps[0:D, 512:1024], wqm[:], xc, start=True, stop=True)
                nc.scalar.activation(
                    E_sb[:, ch * 512:(ch + 1) * 512],
                    eps[0:D, 512:1024],
                    mybir.ActivationFunctionType.Tanh,
                    bias=ce[:, 0:1],
                    scale=1.0,
                )

            # Vt[n, c] = sum_ch x[ch, n] * Wv[c, ch]  (V transposed, raw)
            for ch in range(n_tok // 512):
                vps = psum_e.tile([128, GJT * 512], F32, tag="eps")
                for k in range(4):
                    jt = ch * 4 + k
                    nc.tensor.matmul(
                        vps[:, k * 128:(k + 1) * 128],
                        x_bf[:, jt * 128:(jt + 1) * 128],
                        wvT[:],
                        start=True,
                        stop=True,
                    )
                nc.vector.tensor_copy(Vt_sb[:, ch * 512:(ch + 1) * 512], vps[:, 0:512])

            # ---- main attention loop ----
            for ic in range(n_ic):
                i0 = ic * IC
                out_ps = psum_o.tile([128, IC], F32, tag="o")
                s_ps = psum_s.tile([1, IC], F32, tag="s")
                T0 = acc.tile([128, IC], BF16, tag="T0")
                T1 = acc.tile([128, IC], BF16, tag="T1")
                nc.vector.memset(T0[:], 0.0)
                nc.vector.memset(T1[:], 0.0)

                for g in range(n_g):
                    eps = psum_e.tile([128, GJT * 512], F32, tag="eps")
                    xp = work.tile([128, GJT * 512], BF16, tag="xp")
                    for r in range(GJT):
                        jt = g * GJT + r
                        # energyP[j, i] = sum_e K[e, j] * E[e, i]
                        nc.tensor.matmul(
                            eps[:, r * 512:(r + 1) * 512],
                            K_sb[:, jt * 128:(jt + 1) * 128],
                            E_sb[:, i0:i0 + IC],
                            start=True,
                            stop=True,
                        )
                    nc.scalar.activation(
                        xp[:], eps[:], mybir.ActivationFunctionType.Exp
                    )
                    for r in range(GJT):
                        jt = g * GJT + r
                        # out_raw[c, i] += sum_j Vt[j, c] * expP[j, i]
                        nc.tensor.matmul(
                            out_ps[:],
                            Vt_sb[:, jt * 128:(jt + 1) * 128],
                            xp[:, r * 512:(r + 1) * 512],
                            start=(g == 0 and r == 0),
                            stop=(g == n_g - 1 and r == GJT - 1),
                        )
                        # denominator partial sums (bf16)
                        Tg = T0 if r == 0 else T1
                        nc.vector.tensor_add(
                            Tg[:], Tg[:], xp[:, r * 512:(r + 1) * 512]
                        )

                # S[i] = sum_j T0[j, i] + T1[j, i] via ones matmul
                nc.tensor.matmul(s_ps[:], ones_bf[:], T0[:], start=True, stop=False)
                nc.tensor.matmul(s_ps[:], ones_bf[:], T1[:], start=False, stop=True)

                sr = small.tile([1, IC], F32, tag="sr")
                nc.vector.reciprocal(sr[:], s_ps[:])
                nc.vector.tensor_scalar_mul(sr[:], sr[:], gamma[:, 0:1])
                bc = small.tile([128, IC], F32, tag="bc")
                sr_ap = sr[:]
                sr_bcast = bass.AP(
                    tensor=sr_ap.tensor,
                    offset=sr_ap.offset,
                    ap=[[1, 1], [0, 128], [1, IC]],
                )
                nc.gpsimd.dma_start(bc[:], sr_bcast)
                # z = out_raw * (gamma / S)
                z = small.tile([128, IC], F32, tag="z")
                nc.vector.scalar_tensor_tensor(
                    z[:], out_ps[:], 0.0, bc[:],
                    op0=mybir.AluOpType.add, op1=mybir.AluOpType.mult,
                )
                # y = z + gamma*bv + x
                zf = small.tile([128, IC], F32, tag="zf")
                nc.vector.scalar_tensor_tensor(
                    zf[:], z[:], gbv[:, 0:1], xt[:, i0:i0 + IC],
                    op0=mybir.AluOpType.add, op1=mybir.AluOpType.add,
                )
                nc.sync.dma_start(out_d[:, i0:i0 + IC], zf[:])

    nc.compile()
    return nc


_NC_CACHE: dict = {}


def _get_nc(n_tok: int = N):
    if n_tok not in _NC_CACHE:
        _NC_CACHE[n_tok] = build_nc(n_tok)
    return _NC_CACHE[n_tok]


def make_in_maps(x, Wq, bq, Wk, bk, Wv, bv, Wm, b, gamma, n_tok: int = N):
    """Host-side prep: tiny weight transforms + per-core sharding (batch b -> core b)."""
    x = np.ascontiguousarray(np.asarray(x, np.float32)).reshape(B, C, n_tok)
    Wq = np.asarray(Wq, np.float32)
    Wk = np.asarray(Wk, np.float32)
    Wv = np.asarray(Wv, np.float32)
    Wm = np.asarray(Wm, np.float32)
    bqv = np.asarray(bq, np.float32)
    bv_ = np.asarray(bv, np.float32)
    bs = np.asarray(b, np.float32)
    gm = np.asarray(gamma, np.float32)

    import ml_dtypes
    bf16 = ml_dtypes.bfloat16
    wkT = np.ascontiguousarray(Wk.T).astype(bf16)       # [C, D]
    wqm = np.ascontiguousarray(Wq.T @ Wm).astype(bf16)  # [C, D]
    wvT = np.ascontiguousarray(Wv.T).astype(bf16)       # [C, C]
    ce = (Wm.T @ bqv + bs[0]).reshape(D, 1)             # [D, 1]
    gmat = gm.reshape(1, 1)
    gbv = (gm[0] * bv_).reshape(C, 1)

    common = {
        "wkT": wkT, "wqm": wqm, "wvT": wvT,
        "ce": np.ascontiguousarray(ce, np.float32),
        "gamma": np.ascontiguousarray(gmat, np.float32),
        "gbv": np.ascontiguousarray(gbv, np.float32),
    }
    return [{"x": np.ascontiguousarray(x[core]), **common} for core in range(B)]


def kernel(**inputs) -> np.ndarray:
    nc = _get_nc(N)
    in_maps = make_in_maps(**inputs)
    res = bass_utils.run_bass_kernel_spmd(
        nc, in_maps, core_ids=list(range(N_CORES))
    )
    out = np.stack([res.results[core]["out"] for core in range(B)])
    return out.reshape(B, C, HH, WW).astype(np.float32)
